# revision 26
# baseline (speedup 1.0000x reference)
"""Causal multi-head attention (RoPE) TRN2 Bass kernel — deferred-pipeline v2.

Problem: x[2,2048,2048] fp32, Wq/Wk/Wv/Wo [2048,2048], 16 heads, d_k=128,
causal softmax attention with interleaved RoPE, out = attn_out @ Wo.

Sharding (8 cores): core = b*4 + g handles batch b and head group g
(4 heads = 512 feature columns). Wq/Wk/Wv split column-wise, Wo row-wise;
the "all-reduce" after the output projection is done on the host by summing
the 4 partial outputs per batch (gather/unshard step).

v2 structure (vs the 338us v1): the whole kernel is ONE software-pipelined
PE stream.  Section S_c interleaves, at proj-group granularity:
  - projections of chunk c (Q/K via lhsT=W tiles + fused RoPE; V natural
    layout, m-sequential for c>=1),
  - the causal attention of chunk c-1 (deferred one full chunk so its exps
    and fp8 copies spread over the projection span instead of saturating
    ScalarE in a packed attention phase),
  - the output projection of chunk c-2.
Tail: attn(3) + wo(2) interleaved, then wo(3) with quartered final DMAs.

Row-sums (softmax denominators) use fp8e4 DoubleRow matmuls: pairs of
128-key blocks are contracted in one pass (2x PE rate), with out partitions
64 (two heads share one PSUM bank at partition offsets 0/64).  exp runs
with bias=-ln(16) so e is pre-scaled by 1/16 (fp8 range headroom: raw e can
reach e^6.4 > 240 = fp8e4 max); the 1/16 cancels between o and rs in the
normalization, so no compensation op exists.  Diagonal blocks' masked
column prefixes live in persistent SBUF fp8 slots zeroed once at startup
and never overwritten (copies only touch [lo:]), so pair sums see exact
zeros with no recurring zero-fill.  Denominator fp8 error averages down
~1/sqrt(n_keys); worst case (first rows) dilutes 4x across 16 heads.

Engine assignment (balanced against per-section PE spans):
  ScalarE: exps (+ half the wo PSUM->SBUF copies in the tail)
  VectorE: RoPE rotate-copies/muls, norms (reciprocal+broadcast+mul),
           most wo copies
  GpSimd:  V-projection PSUM->SBUF copies, causal-mask muls, all
           bf16->fp8 e copies (pair-granular for history blocks)
Startup: input feed split across the two HW DMA queues (sync + scalar);
PE pre-warm matmuls (operands memset on GpSimd) lift the HAM clock gate
during the DMA wait.

RoPE pair trick: scores are invariant under any permutation of d_k applied
to both Q and K, so W columns are permuted per head to [even..., odd...] on
the host; the rotate pairs then live 64 partitions apart (two plain
partition-offset copies instead of an interleaved shuffle), and cosT/sinT
are permuted/sign-baked to match.
"""

import math
import sys

sys.path.insert(0, "/opt/trn_rl_repo")

import ml_dtypes
import numpy as np

D_MODEL = 2048
SEQ = 2048
BATCH = 2
N_CORES = 8
HEADS_PER_CORE = 4
GCOLS = HEADS_PER_CORE * 128  # 512 feature columns per core
KB = D_MODEL // 128  # 16 contraction blocks
N_CHUNKS = SEQ // 512  # 4
SCALE = 1.0 / math.sqrt(128.0)
LN16 = math.log(16.0)
LAG = 6

_CACHE = {}


def _build_program():
    import concourse.mybir as mybir
    import concourse.tile as tile
    from concourse import bacc

    F = mybir.dt.float32
    BF = mybir.dt.bfloat16
    F8 = mybir.dt.float8e4
    AF = mybir.ActivationFunctionType
    DR = mybir.MatmulPerfMode.DoubleRow

    nc = bacc.Bacc("TRN2", target_bir_lowering=False, debug=False,
                   num_devices=N_CORES)

    xT_d = nc.dram_tensor("xT", (D_MODEL, SEQ), BF, kind="ExternalInput").ap()
    Wq_d = nc.dram_tensor("Wq", (D_MODEL, GCOLS), BF, kind="ExternalInput").ap()
    Wk_d = nc.dram_tensor("Wk", (D_MODEL, GCOLS), BF, kind="ExternalInput").ap()
    Wv_d = nc.dram_tensor("Wv", (D_MODEL, GCOLS), BF, kind="ExternalInput").ap()
    Wo_d = nc.dram_tensor("Wo", (GCOLS, D_MODEL), BF, kind="ExternalInput").ap()
    cs_d = nc.dram_tensor("cs", (N_CHUNKS * 128, 1024), mybir.dt.float16,
                          kind="ExternalInput").ap()
    mask_d = nc.dram_tensor("mask", (128, 896), BF, kind="ExternalInput").ap()
    out_d = nc.dram_tensor("out", (SEQ, D_MODEL), BF,
                           kind="ExternalOutput").ap()

    with tile.TileContext(nc) as tc:
        with tc.tile_pool(name="resid", bufs=1) as resid, \
             tc.tile_pool(name="xtp", bufs=2) as xtp, \
             tc.tile_pool(name="csp", bufs=2) as csp, \
             tc.tile_pool(name="qtp", bufs=2) as qtp, \
             tc.tile_pool(name="otp", bufs=2) as otp, \
             tc.tile_pool(name="ep", bufs=6) as ep, \
             tc.tile_pool(name="epd", bufs=6) as epd, \
             tc.tile_pool(name="e8p", bufs=8) as e8p, \
             tc.tile_pool(name="ropep", bufs=2) as ropep, \
             tc.tile_pool(name="rcp", bufs=1) as rcp, \
             tc.tile_pool(name="outp", bufs=2) as outp, \
             tc.tile_pool(name="psA", bufs=4, space="PSUM") as psA, \
             tc.tile_pool(name="psB", bufs=4, space="PSUM") as psB:

            # ones for the fp8 DoubleRow row-sum lhsT and the PE warm-up;
            # prepared on GpSimd so warm-up starts as soon as engines are up.
            ones8 = resid.tile([128, 2, 64], F8, tag="ones8")
            nc.gpsimd.memset(ones8[:], 1.0)
            ones_bf = resid.tile([128, 128], BF, tag="ones_bf")
            nc.gpsimd.memset(ones_bf[:], 1.0)
            # exp bias AP: -ln(16) pre-scale (see module docstring)
            ebias = resid.tile([128, 1], F, tag="ebias")
            nc.gpsimd.memset(ebias[:], -LN16)

            # PE pre-warm during the initial DMA wait (HAM clock gate needs
            # ~4096 busy cycles to lift the PE from 1.2 to 2.4 GHz).
            warm_ps = psA.tile([128, 128], F, tag="flow", name="warm")
            for _ in range(28):
                nc.tensor.matmul(warm_ps[:], ones_bf[:], ones_bf[:],
                                 start=True, stop=True)

            mask_sb = resid.tile([128, 128], BF, tag="mask")
            # persistent-zero diag fp8 slots: slot (h%2)*4+d is only ever
            # written at columns [128d:], so the masked prefix stays zero.
            e8diag = resid.tile([128, 8, 512], F8, tag="e8diag")
            nc.gpsimd.memset(e8diag[:], 0.0)

            KT = [resid.tile([128, HEADS_PER_CORE, 512], BF, tag=f"KT{c}",
                             name=f"KT{c}") for c in range(N_CHUNKS)]
            V = [resid.tile([128, HEADS_PER_CORE, GCOLS], BF, tag=f"V{c}",
                            name=f"V{c}") for c in range(N_CHUNKS)]
            wo = resid.tile([128, HEADS_PER_CORE, D_MODEL], BF, tag="wo")
            wq = resid.tile([128, KB, GCOLS], BF, tag="wq")
            wk = resid.tile([128, KB, GCOLS], BF, tag="wk")
            wv = resid.tile([128, KB, GCOLS], BF, tag="wv")

            xT_r = xT_d.rearrange("(ko p) s -> p ko s", p=128)
            cs_r = cs_d.rearrange("(j p) s -> p j s", p=128)
            Wq_r = Wq_d.rearrange("(ko p) m -> p ko m", p=128)
            Wk_r = Wk_d.rearrange("(ko p) m -> p ko m", p=128)
            Wv_r = Wv_d.rearrange("(ko p) m -> p ko m", p=128)
            Wo_r = Wo_d.rearrange("(c p) n -> p c n", p=128)

            # ---------- wo emitter ----------
            class WoEmitter:
                def __init__(self, w, prev_ot, tail=False):
                    self.w = w
                    self.ot = prev_ot
                    self.t = 0
                    self.obw = None
                    self.tail = tail

                def emit(self, nsteps):
                    for _ in range(nsteps):
                        t = self.t
                        self.t += 1
                        m, n = t // 4, t % 4
                        alt = self.tail and (t % 2 == 1)
                        pool = psB if (self.tail and t % 2) else psA
                        ps = pool.tile([128, 512], F,
                                       tag="flow" if pool is psA else "hold",
                                       name="wops")
                        for c in range(HEADS_PER_CORE):
                            nc.tensor.matmul(
                                ps[:], self.ot[:, c, m * 128:(m + 1) * 128],
                                wo[:, c, n * 512:(n + 1) * 512],
                                start=(c == 0), stop=(c == 3),
                                skip_group_check=True)
                        if n == 0:
                            self.obw = outp.tile([128, D_MODEL], BF,
                                                 tag="obw", name="obw")
                        if alt:
                            nc.scalar.copy(self.obw[:, n * 512:(n + 1) * 512],
                                           ps[:])
                        else:
                            nc.vector.tensor_copy(
                                self.obw[:, n * 512:(n + 1) * 512], ps[:])
                        row = (4 * self.w + m) * 128
                        if self.tail and m == 3:
                            # final band: ship each quarter as soon as its
                            # copy lands so the last transfer is small
                            nc.scalar.dma_start(
                                out_d[row:row + 128, n * 512:(n + 1) * 512],
                                self.obw[:, n * 512:(n + 1) * 512])
                        elif n == 3:
                            nc.scalar.dma_start(out_d[row:row + 128, :],
                                                self.obw[:])

            # ---------- attention emitter (for chunk a, deferred) ----------
            class AttnEmitter:
                def __init__(self, a, qt, ot):
                    self.a = a
                    self.qt = qt
                    self.ot = ot
                    diag = list(range(4 * a, 4 * a + 4))
                    hist = list(range(4 * a))
                    if hist:
                        kb_order = []
                        step = max(1, len(hist) // 4)
                        hi = 0
                        for dkb in diag:
                            kb_order.append(dkb)
                            kb_order.extend(hist[hi:hi + step])
                            hi += step
                        kb_order.extend(hist[hi:])
                    else:
                        kb_order = diag
                    self.kb_order = kb_order
                    self.nkb = len(kb_order)
                    self.blocks = [(h, i) for h in range(HEADS_PER_CORE)
                                   for i in range(self.nkb)]
                    self.pos = 0
                    self.pend = []
                    self.pairinfo = self._pair_schedule()
                    self.o_tiles = {}
                    self.rs_tiles = {}
                    self.ecur = {}       # h -> current hist bf16 pair tile
                    self.e8cur = {}      # h -> current hist fp8 pair tile
                    self.e8fired = {}    # (h, pid) -> fp8 pair tile

                def _pair_schedule(self):
                    # Map block index i -> (pair_id, half).  Diag d pairs:
                    # (0,1)->('d',0), (2,3)->('d',1); hist pairs ('h',k) in
                    # kb_order arrival order.  Same for every head.
                    info = {}
                    hist_count = 0
                    pair_members = {}
                    for i, kb in enumerate(self.kb_order):
                        d = kb - 4 * self.a
                        if d >= 0:
                            pid, half = ("d", d // 2), d % 2
                        else:
                            pid, half = ("h", hist_count // 2), hist_count % 2
                            hist_count += 1
                        info[i] = (pid, half)
                        pair_members.setdefault(pid, []).append(i)
                    # n-tile contributions: n covered iff some member's live
                    # range [lo:512) intersects cols [n*256:(n+1)*256)
                    contrib = {}
                    for pid, mem in pair_members.items():
                        los = []
                        for i in mem:
                            d = self.kb_order[i] - 4 * self.a
                            los.append(128 * d if d > 0 else 0)
                        contrib[pid] = [n for n in (0, 1)
                                        if min(los) < (n + 1) * 256]
                    # all rs matmuls of a head are emitted together at head
                    # end (their only consumer is the norm), in this order
                    pair_order = sorted(pair_members.keys())
                    return dict(info=info, contrib=contrib,
                                pair_order=pair_order)

                def emit(self, nblocks):
                    for _ in range(nblocks):
                        if self.pos >= len(self.blocks):
                            return
                        h, i = self.blocks[self.pos]
                        self.pos += 1
                        self._emit_block(h, i)

                def _emit_block(self, h, i):
                    a = self.a
                    kb = self.kb_order[i]
                    d = kb - 4 * a
                    lo = 128 * d if d > 0 else 0
                    if i == 0:
                        self.o_tiles[h] = psB.tile([128, 512], F, tag="hold",
                                                   name=f"o{a}_{h}")
                        # rs tile reused sequentially across heads (start
                        # flags re-zero it); fp8 path writes only [0:64]
                        # (DoubleRow dst must start at PSUM partition 0).
                        self.rs_tiles[h] = psB.tile([128, 512], F, tag="hold",
                                                    name=f"rs{a}_{h}")
                    s_ps = psA.tile([128, 512], F, tag="flow")
                    nc.tensor.matmul(
                        s_ps[:, lo:],
                        KT[kb // 4][:, h, (kb % 4) * 128:(kb % 4 + 1) * 128],
                        self.qt[:, h, lo:], start=True, stop=True,
                        skip_group_check=True)
                    while len(self.pend) >= LAG:
                        self._flush_one()
                    pid, half = self.pairinfo["info"][i]
                    use_f8 = a > 0
                    if pid[0] == "d":
                        # exp with the 1/16 pre-scale folded into the bias
                        e = epd.tile([128, 512], BF, tag="ed")
                        nc.scalar.activation(e[:, lo:], s_ps[:, lo:], AF.Exp,
                                             bias=ebias[:], scale=SCALE)
                        nc.gpsimd.tensor_mul(e[:, lo:lo + 128],
                                             e[:, lo:lo + 128], mask_sb[:])
                        if use_f8:
                            s8 = (h % 2) * 4 + pid[1] * 2 + half
                            nc.vector.tensor_copy(e8diag[:, s8, lo:],
                                                  e[:, lo:])
                        self.pend.append((h, i, kb, e, None, lo))
                    else:
                        if half == 0:
                            self.ecur[h] = ep.tile([128, 2, 512], BF, tag="e",
                                                   name="epair")
                            self.e8cur[h] = e8p.tile([128, 2, 512], F8,
                                                     tag="e8", name="e8pair")
                        e_pair = self.ecur[h]
                        nc.scalar.activation(e_pair[:, half, :], s_ps[:],
                                             AF.Exp, bias=ebias[:],
                                             scale=SCALE)
                        if half == 1:
                            # one pair-granular fp8 copy for both halves
                            nc.vector.tensor_copy(self.e8cur[h][:],
                                                  e_pair[:])
                            self.e8fired[(h, pid)] = self.e8cur[h]
                        self.pend.append((h, i, kb, e_pair, half, lo))

                def _flush_one(self):
                    h, i, kb, e_t, half, lo = self.pend.pop(0)
                    o_ps = self.o_tiles[h]
                    rhs = e_t[:, half, lo:] if half is not None \
                        else e_t[:, lo:]
                    nc.tensor.matmul(
                        o_ps[:, lo:],
                        V[kb // 4][:, kb % 4, h * 128:(h + 1) * 128],
                        rhs, start=(i == 0), stop=(i == self.nkb - 1),
                        skip_group_check=True)
                    if self.a == 0:
                        # bf16 row-sums for chunk 0: its early rows have few
                        # live keys, where fp8 denominator error is worst
                        nc.tensor.matmul(
                            self.rs_tiles[h][:, lo:], ones_bf[:], rhs,
                            start=(i == 0), stop=(i == self.nkb - 1),
                            skip_group_check=True)
                    if i == self.nkb - 1:
                        if self.a > 0:
                            self._emit_rs(h)
                        self._emit_norm(h)

                def _emit_rs(self, h):
                    # all fp8 DoubleRow row-sum matmuls for head h, emitted
                    # at head end: every e8 copy is long done, and a single
                    # start=True zeroes the whole PSUM bank (zero-region =
                    # bank), so only the first matmul carries it.
                    pi = self.pairinfo
                    rs = self.rs_tiles[h]
                    todo = [(pid, n) for pid in pi["pair_order"]
                            for n in pi["contrib"][pid]]
                    for idx, (pid, n) in enumerate(todo):
                        if pid[0] == "d":
                            s0 = (h % 2) * 4 + pid[1] * 2
                            e8ap = e8diag[:, s0:s0 + 2,
                                          n * 256:(n + 1) * 256]
                        else:
                            e8ap = self.e8fired[(h, pid)][
                                :, :, n * 256:(n + 1) * 256]
                        nc.tensor.matmul(
                            rs[0:64, n * 256:(n + 1) * 256],
                            ones8[:], e8ap,
                            start=(idx == 0), stop=(idx == len(todo) - 1),
                            perf_mode=DR, skip_group_check=True)

                def _emit_norm(self, h):
                    o_ps = self.o_tiles[h]
                    rs = self.rs_tiles[h]
                    rc = rcp.tile([128, 512], F, tag="rc")
                    if self.a == 0:
                        nc.vector.reciprocal_approx_fast(rc[:], rs[:])
                    else:
                        nc.vector.reciprocal_approx_fast(rc[0:64, :],
                                                         rs[0:64, :])
                        nc.vector.tensor_copy(rc[64:128, :], rc[0:64, :])
                    nc.vector.tensor_mul(self.ot[:, h, :], o_ps[:], rc[:])

                def finish(self):
                    self.emit(len(self.blocks) - self.pos)
                    while self.pend:
                        self._flush_one()

            # ---------- input staging ----------
            def stage_inputs(jj):
                sl = slice(jj * 512, (jj + 1) * 512)
                xt_n = xtp.tile([128, KB, 512], BF, tag="xt", name="xt_n")
                nc.sync.dma_start(xt_n[:, 0:8], xT_r[:, 0:8, sl])
                nc.sync.dma_start(xt_n[:, 8:KB], xT_r[:, 8:KB, sl])
                cs_n = csp.tile([128, 1024], mybir.dt.float16, tag="cs",
                                name="cs_n")
                nc.sync.dma_start(cs_n[:], cs_r[:, jj, :])
                return xt_n, cs_n[:, 0:512], cs_n[:, 512:1024]

            # ---------- rope ----------
            def emit_rope(ps, out_ap, cos_t, sin_t):
                rot = ropep.tile([128, 512], F, tag="rot")
                nc.vector.tensor_copy(rot[:64, :], ps[64:128, :])
                nc.vector.tensor_copy(rot[64:128, :], ps[:64, :])
                nc.vector.tensor_mul(out_ap, ps[:], cos_t[:])
                nc.vector.tensor_mul(rot[:], rot[:], sin_t[:])
                nc.vector.tensor_add(out_ap, out_ap, rot[:])

            staged = {}
            qts = {}
            ots = {}
            attn = None
            woe = None
            wo_stagger = (0, 3, 3, 2, 2, 2, 2, 2)

            for c in range(N_CHUNKS):
                ssl = slice(c * 512, (c + 1) * 512)
                if c == 0:
                    # --- chunk-0 dual-queue feed.  Q/K weights are COLUMN
                    # (head) sliced: proj group g=m consumes ALL 16 k-slabs
                    # of head m's 128 columns, so a 0.5MB column slice
                    # unblocks a whole group; k-sliced slabs would make
                    # group 0 wait for 1.5MB+ and stall the HAM clock lift.
                    # three queues (~100GB/s each), balanced ~2.8MB apiece:
                    # sync: xt-even + wk[0:8]; scalar: wq + xt-odd + cs;
                    # gpsimd (software DGE): wv + wk[8:16]
                    xt = xtp.tile([128, KB, 512], BF, tag="xt")
                    nc.sync.dma_start(mask_sb[:], mask_d[:, 384:512])
                    nc.scalar.dma_start(wq[:, 0:2], Wq_r[:, 0:2, 0:GCOLS])
                    nc.sync.dma_start(xt[:, 0:2], xT_r[:, 0:2, ssl])
                    nc.scalar.dma_start(wq[:, 2:4], Wq_r[:, 2:4, 0:GCOLS])
                    nc.sync.dma_start(xt[:, 2:4], xT_r[:, 2:4, ssl])
                    cs_t = csp.tile([128, 1024], mybir.dt.float16, tag="cs")
                    nc.sync.dma_start(cs_t[:], cs_r[:, 0, :])
                    nc.gpsimd.dma_start(wv[:, 0:2], Wv_r[:, 0:2, 0:GCOLS])
                    cos_t, sin_t = cs_t[:, 0:512], cs_t[:, 512:1024]
                    nc.scalar.dma_start(wq[:, 4:8], Wq_r[:, 4:8, 0:GCOLS])
                    nc.sync.dma_start(xt[:, 4:6], xT_r[:, 4:6, ssl])
                    nc.scalar.dma_start(xt[:, 6:8], xT_r[:, 6:8, ssl])
                    nc.gpsimd.dma_start(wv[:, 2:6], Wv_r[:, 2:6, 0:GCOLS])
                    nc.scalar.dma_start(wq[:, 8:12], Wq_r[:, 8:12, 0:GCOLS])
                    nc.sync.dma_start(xt[:, 8:10], xT_r[:, 8:10, ssl])
                    nc.scalar.dma_start(xt[:, 10:12], xT_r[:, 10:12, ssl])
                    nc.scalar.dma_start(wq[:, 12:KB], Wq_r[:, 12:KB, 0:GCOLS])
                    nc.sync.dma_start(xt[:, 12:14], xT_r[:, 12:14, ssl])
                    nc.scalar.dma_start(xt[:, 14:KB], xT_r[:, 14:KB, ssl])
                    nc.gpsimd.dma_start(wv[:, 6:11], Wv_r[:, 6:11, 0:GCOLS])
                    nc.sync.dma_start(wk[:, 0:4], Wk_r[:, 0:4, 0:GCOLS])
                    nc.sync.dma_start(wk[:, 4:8], Wk_r[:, 4:8, 0:GCOLS])
                    nc.gpsimd.dma_start(wv[:, 11:KB], Wv_r[:, 11:KB, 0:GCOLS])
                    nc.gpsimd.dma_start(wk[:, 8:12], Wk_r[:, 8:12, 0:GCOLS])
                    nc.gpsimd.dma_start(wk[:, 12:KB], Wk_r[:, 12:KB, 0:GCOLS])
                else:
                    xt, cos_t, sin_t = staged.pop(c)
                if c == 0:
                    staged[1] = stage_inputs(1)
                qt = qtp.tile([128, HEADS_PER_CORE, 512], BF, tag="qt")
                qts[c] = qt
                if c >= 1:
                    ots[c - 1] = otp.tile([128, HEADS_PER_CORE, 512], BF,
                                          tag="ot", name=f"ot{c-1}")
                    attn = AttnEmitter(c - 1, qts.pop(c - 1), ots[c - 1])
                if c >= 2:
                    woe = WoEmitter(c - 2, ots.pop(c - 2))

                quotas = {0: (0,) * 8,
                          1: (2,) * 8,
                          2: (4,) * 8,
                          3: (6,) * 8}[c]
                groups = [(qt, True, wq, m) for m in range(HEADS_PER_CORE)]
                groups += [(KT[c], False, wk, m)
                           for m in range(HEADS_PER_CORE)]

                if c == 0:
                    # chunk-0: V k-steps threaded into the DMA wait windows;
                    # 4 vps tiles accumulate across all groups
                    vps = [psB.tile([128, 512], F, tag="hold",
                                    name=f"vps{m}") for m in range(4)]
                    for g, (dst, is_q, w, m) in enumerate(groups):
                        ps = psA.tile([128, 512], F, tag="flow")

                        def qk_half(lo_, hi_):
                            for k in range(lo_, hi_):
                                nc.tensor.matmul(
                                    ps[:], w[:, k, m * 128:(m + 1) * 128],
                                    xt[:, k], start=(k == 0),
                                    stop=(k == KB - 1))

                        def v_steps():
                            for k in (2 * g, 2 * g + 1):
                                for m2 in range(4):
                                    nc.tensor.matmul(
                                        vps[m2][:],
                                        xt[:, k, m2 * 128:(m2 + 1) * 128],
                                        wv[:, k],
                                        start=(k == 0), stop=(k == KB - 1))

                        if g < 2:
                            qk_half(0, 8)
                            v_steps()
                            qk_half(8, KB)
                        else:
                            qk_half(0, KB)
                            v_steps()
                        emit_rope(ps, dst[:, m, :], cos_t, sin_t)
                    for m in range(4):
                        nc.scalar.copy(V[0][:, m, :], vps[m][:])
                    nc.sync.dma_start(wo[:], Wo_r)
                else:
                    vps_cur = {}
                    for g, (dst, is_q, w, m) in enumerate(groups):
                        ps = psA.tile([128, 512], F, tag="flow")
                        for k in range(KB):
                            nc.tensor.matmul(
                                ps[:], w[:, k, m * 128:(m + 1) * 128],
                                xt[:, k], start=(k == 0), stop=(k == KB - 1),
                                skip_group_check=True)
                        # V projection, m-sequential: group g covers
                        # seq-subtile vm = g//2, k-half g%2
                        vm, khalf = g // 2, g % 2
                        if khalf == 0:
                            vps_cur[vm] = psB.tile([128, 512], F, tag="hold",
                                                   name=f"vps{vm}")
                        for k in range(8 * khalf, 8 * khalf + 8):
                            nc.tensor.matmul(
                                vps_cur[vm][:],
                                xt[:, k, vm * 128:(vm + 1) * 128],
                                wv[:, k], start=(k == 0), stop=(k == KB - 1),
                                skip_group_check=True)
                        if khalf == 1:
                            nc.scalar.copy(V[c][:, vm, :],
                                           vps_cur.pop(vm)[:])
                        if woe is not None:
                            woe.emit(wo_stagger[g])
                        emit_rope(ps, dst[:, m, :], cos_t, sin_t)
                        attn.emit(quotas[g])
                    attn.finish()

                if 0 < c < N_CHUNKS - 1:
                    staged[c + 1] = stage_inputs(c + 1)

            # ---------- tail: attn(3) + wo(2), then wo(3) ----------
            ots[3] = otp.tile([128, HEADS_PER_CORE, 512], BF, tag="ot",
                              name="ot3")
            attn = AttnEmitter(3, qts.pop(3), ots[3])
            woe = WoEmitter(2, ots.pop(2))
            for _ in range(16):
                attn.emit(4)
                woe.emit(1)
            attn.finish()
            woe_f = WoEmitter(3, ots.pop(3), tail=True)
            woe_f.emit(16)

    nc.compile()
    return nc


def _get_program():
    if "nc" not in _CACHE:
        _CACHE["nc"] = _build_program()
    return _CACHE["nc"]


def _host_prep(x, token_positions, Wq, Wk, Wv, Wo):
    bf16 = ml_dtypes.bfloat16
    x = np.asarray(x, dtype=np.float32)
    Wq = np.asarray(Wq, dtype=np.float32)
    Wk = np.asarray(Wk, dtype=np.float32)
    Wv = np.asarray(Wv, dtype=np.float32)
    Wo = np.asarray(Wo, dtype=np.float32)
    pos = np.asarray(token_positions).astype(np.float64)

    # RoPE tables in permuted (half-split) layout, transposed to [d_k, s].
    inv = 10000.0 ** (-2.0 * np.arange(64, dtype=np.float64) / 128.0)
    ang = inv[:, None] * pos[None, :]  # [64, S]
    cos_h = np.cos(ang)
    sin_h = np.sin(ang)
    cosT = np.concatenate([cos_h, cos_h], axis=0).astype(np.float32)
    sinT = np.concatenate([-sin_h, sin_h], axis=0).astype(np.float32)
    # merged per-chunk [cos512 | sin512] blocks, one DMA per chunk
    cs = np.concatenate(
        [np.concatenate([cosT[:, j * 512:(j + 1) * 512],
                         sinT[:, j * 512:(j + 1) * 512]], axis=1)
         for j in range(SEQ // 512)], axis=0)
    cs = np.ascontiguousarray(cs)

    # half-split permutation of each head's 128 feature columns
    perm = np.concatenate([np.arange(0, 128, 2), np.arange(1, 128, 2)])

    # causal mask strip: mask[p, g] = 1 iff p <= g - 384; diagonal block d
    # (d = kb - 4j) uses columns [384-128d, 896-128d).
    mask = (np.arange(128)[:, None] <= np.arange(896)[None, :] - 384)
    mask = np.ascontiguousarray(mask.astype(bf16))

    def permute_cols(W):  # [2048, 512] -> per-head column permutation
        return np.ascontiguousarray(
            W.reshape(D_MODEL, HEADS_PER_CORE, 128)[:, :, perm].reshape(
                D_MODEL, GCOLS).astype(bf16))

    in_maps = []
    for core in range(N_CORES):
        b, g = divmod(core, 4)
        cols = slice(g * GCOLS, (g + 1) * GCOLS)
        in_maps.append({
            "xT": np.ascontiguousarray(x[b].T.astype(bf16)),
            "Wq": permute_cols(Wq[:, cols]),
            "Wk": permute_cols(Wk[:, cols]),
            "Wv": np.ascontiguousarray(Wv[:, cols].astype(bf16)),
            "Wo": np.ascontiguousarray(Wo[cols, :].astype(bf16)),
            "cs": cs.astype(np.float16),
            "mask": mask,
        })
    return in_maps


def run_sharded(x, token_positions, Wq, Wk, Wv, Wo, trace=False, tmpdir=None):
    """Run the SPMD kernel; returns (full_output, BassKernelResults)."""
    from concourse import bass_utils

    nc = _get_program()
    in_maps = _host_prep(x, token_positions, Wq, Wk, Wv, Wo)
    kwargs = {}
    if trace:
        kwargs = {"trace": True, "tmpdir": tmpdir}
    res = bass_utils.run_bass_kernel_spmd(
        nc, in_maps, core_ids=list(range(N_CORES)), **kwargs)
    out = np.empty((BATCH, SEQ, D_MODEL), dtype=np.float32)
    for b in range(BATCH):
        acc = np.zeros((SEQ, D_MODEL), dtype=np.float64)
        for g in range(4):
            acc += res.results[b * 4 + g]["out"].astype(np.float32)
        out[b] = acc.astype(np.float32)
    return out, res


def kernel(x, token_positions, Wq, Wk, Wv, Wo):
    out, _ = run_sharded(x, token_positions, Wq, Wk, Wv, Wo)
    return out
